# revision 24
# baseline (speedup 1.0000x reference)
"""AdditiveAttention Trainium2 kernel (8 NeuronCores, SPMD data-parallel).

reference:
    q = query @ Wq; k = key @ Wk
    scores[b,q,k] = sum_h wv[h] * tanh(q[b,q,h] + k[b,k,h])   (masked k >= valid_len)
    out = softmax(scores) @ value

Sharding: core i takes q-rows [32i, 32i+32) of EVERY batch, so all 8 cores run an
identical instruction stream (chunk per batch with that batch's k-extent) and the
per-batch valid_len-dependent work (k <= Kc_b, Kc_b = valid_len rounded up to 32)
is split exactly evenly.

Per-core dataflow (per batch chunk; partition dim = hidden):
    DMA-cast q/key/Wq/Wk to bf16, DMA-transpose -> hT layouts
    PE: qT' = Wq^T qT, kT' = Wk^T kT  (bf16)
    DVE: per q-row tensor_scalar_add (kT' + q-col) -> feat tile [128h, 32q*Kc]
    ACT: tanh in-place (bf16), one instruction per (batch, h-tile)
    PE:  scores row = wv^T @ feat[:, j, :]  M=1 matmuls, col-tiled 4-wide so 4
         q-rows land on PSUM partitions {0,32,64,96} -> 4-lane DVE copies
    softmax: reduce_max(negate) -> exp(bias=-max, accum_out=sum) -> reciprocal -> scale
    PE: transpose attn, context = attnT^T @ value (f32)
"""

import contextlib
import ctypes
import math
import sys
import types

if "/opt/trn_rl_repo" not in sys.path:
    sys.path.insert(0, "/opt/trn_rl_repo")

import numpy as np


def _install_ntff_hook():
    """This image's antenv package lacks axon_hooks; inject an equivalent so
    run_bass_kernel_spmd can trace (BASS_TRACE=1) instead of crashing."""
    if "antenv.axon_hooks" in sys.modules:
        return
    mod = types.ModuleType("antenv.axon_hooks")
    _state = {"hook": None}
    mod.set_axon_ntff_profile_hook = lambda h: _state.__setitem__("hook", h)
    mod.get_axon_ntff_profile_hook = lambda: _state["hook"]
    try:
        import antenv

        antenv.axon_hooks = mod
    except ImportError:
        pass
    sys.modules["antenv.axon_hooks"] = mod

    try:
        lib = ctypes.CDLL("/opt/axon/libaxon_pjrt.so")
    except OSError:
        return
    if not hasattr(lib, "axon_start_nrt_profile"):
        return
    lib.axon_start_nrt_profile.argtypes = [ctypes.POINTER(ctypes.c_int64),
                                           ctypes.c_size_t]
    lib.axon_start_nrt_profile.restype = ctypes.c_int64
    lib.axon_stop_nrt_profile.argtypes = [ctypes.c_char_p]
    lib.axon_stop_nrt_profile.restype = ctypes.c_int64

    @contextlib.contextmanager
    def _hook(output_dir, device_ids):
        import jax

        jax.devices()
        if device_ids:
            ids = (ctypes.c_int64 * len(device_ids))(*device_ids)
            rc = lib.axon_start_nrt_profile(ids, len(device_ids))
        else:
            rc = lib.axon_start_nrt_profile(None, 0)
        if rc != 0:
            raise RuntimeError(f"axon_start_nrt_profile rc={rc}")
        try:
            yield
        finally:
            n = lib.axon_stop_nrt_profile(str(output_dir).encode())
            print(f"profile: {n} file(s) written to {output_dir}", file=sys.stderr)

    mod.set_axon_ntff_profile_hook(_hook)


_install_ntff_hook()

import concourse.bass as bass
import concourse.bacc as bacc
import concourse.tile as tile
from concourse import mybir
from concourse.bass_utils import run_bass_kernel_spmd
from concourse.masks import make_identity

F32 = mybir.dt.float32
BF16 = mybir.dt.bfloat16
AF = mybir.ActivationFunctionType

B, Q, K, H = 4, 256, 512 // 2, 512
P = 128
HT = H // P  # 4 h-tiles
N_CORES = 8
QC = Q // N_CORES  # 32 q rows per (core, batch)
NEG = -1e9


def _build(kcs: tuple[int, ...]):
    """Build + compile the SPMD program for the given per-chunk k extents
    (one chunk per active batch, every extent a multiple of 32, <= 256)."""
    nb = len(kcs)
    R = nb * QC  # total q rows per core

    nc = bacc.Bacc("TRN2", target_bir_lowering=False, debug=False,
                   num_devices=N_CORES)

    q_d = nc.dram_tensor("q", [R, H], F32, kind="ExternalInput").ap()
    wq_d = nc.dram_tensor("wq", [H, H], F32, kind="ExternalInput").ap()
    wk_d = nc.dram_tensor("wk", [H, H], F32, kind="ExternalInput").ap()
    wv_d = nc.dram_tensor("wv", [H], F32, kind="ExternalInput").ap()
    k_d, v_d, m_d = [], [], []
    for i, kc in enumerate(kcs):
        k_d.append(nc.dram_tensor(f"k{i}", [kc, H], F32, kind="ExternalInput").ap())
        v_d.append(nc.dram_tensor(f"v{i}", [kc, H], F32, kind="ExternalInput").ap())
        m_d.append(nc.dram_tensor(f"m{i}", [kc], F32, kind="ExternalInput").ap())
    out_d = nc.dram_tensor("out", [nb, QC, H], F32, kind="ExternalOutput").ap()

    kcmax = max(kcs)

    with tile.TileContext(nc) as tc:
        with (
            tc.tile_pool(name="consts", bufs=1) as consts,
            tc.tile_pool(name="kv", bufs=3) as kv,
            tc.tile_pool(name="feat", bufs=8) as featp,
            tc.tile_pool(name="sm", bufs=2) as sm,
            tc.tile_pool(name="ps_sc", bufs=1, space="PSUM") as ps_scp,
            tc.tile_pool(name="ps_misc", bufs=2, space="PSUM") as ps_miscp,
            tc.tile_pool(name="ps_t", bufs=2, space="PSUM") as ps_tp,
            tc.tile_pool(name="ps_ctx", bufs=2, space="PSUM") as ps_ctxp,
        ):
            # query rows first: everything upstream of the feature adds hangs
            # off the q/key casts, so they lead the SWDGE queue
            q_bf = consts.tile([P, H], BF16)
            nc.gpsimd.dma_start(out=q_bf[:R, :], in_=q_d)

            # weights, cast to bf16 during DMA; layout [p, t, h_out], h_in = t*128+p
            wk_bf = consts.tile([P, HT, H], BF16)
            nc.gpsimd.dma_start(out=wk_bf, in_=wk_d.rearrange("(t p) o -> p t o", p=P))
            wq_bf = consts.tile([P, HT, H], BF16)
            nc.gpsimd.dma_start(out=wq_bf, in_=wq_d.rearrange("(t p) o -> p t o", p=P))

            ident_bf = consts.tile([P, P], BF16)
            make_identity(nc, ident_bf)

            wv_f = consts.tile([P, HT], F32)
            nc.sync.dma_start(out=wv_f, in_=wv_d.rearrange("(t p) -> p t", p=P))
            wv_bf = consts.tile([P, HT], BF16)
            nc.vector.tensor_copy(wv_bf, wv_f)
            # wvZ[:, t, r, :] = wv tile t in column r, zeros elsewhere: an M<=4
            # score matmul with this lhsT adds row-pack r's scores at PSUM
            # partition base+r and zeros into the others (which accumulate away)
            wvZ = consts.tile([P, HT, 4, 4], BF16)
            nc.vector.memset(wvZ, 0.0)
            for t in range(HT):
                for r in range(4):
                    nc.vector.tensor_copy(wvZ[:, t, r, r:r + 1], wv_bf[:, t:t + 1])

            # qT via PE transpose (PE is idle early; DMA-transpose costs ~1.2us
            # apiece on the sync ring and serializes the startup)
            qT_bf = consts.tile([P, HT, R], BF16)
            for t in range(HT):
                ps = ps_tp.tile([P, P], BF16, tag="ps_t")
                nc.tensor.transpose(ps[:, :R], q_bf[:R, t * P:(t + 1) * P],
                                    ident_bf[:R, :R])
                nc.scalar.copy(qT_bf[:, t, :], ps[:, :R])

            # qproj[p, t_out, r] (f32): h_out = t_out*128+p
            qproj = consts.tile([P, HT, R], F32)
            for to in range(HT):
                ps = ps_miscp.tile([P, 512], F32, tag="ps_misc")
                for ti in range(HT):
                    nc.tensor.matmul(ps[:, :R], lhsT=wq_bf[:, ti, to * P:(to + 1) * P],
                                     rhs=qT_bf[:, ti, :],
                                     start=(ti == 0), stop=(ti == HT - 1))
                nc.vector.tensor_copy(qproj[:, to, :], ps[:, :R])

            for i, kc in enumerate(kcs):
                nkt = math.ceil(kc / P)
                rows = [min(P, kc - kt * P) for kt in range(nkt)]

                key_bf = kv.tile([P, nkt, H], BF16, tag="key")
                for kt in range(nkt):
                    r0 = kt * P
                    nc.gpsimd.dma_start(out=key_bf[:rows[kt], kt, :],
                                        in_=k_d[i][r0:r0 + rows[kt], :])

                # keyT via PE transpose + ACT copy (see qT note)
                keyT_bf = kv.tile([P, HT, kc], BF16, tag="keyT")
                for kt in range(nkt):
                    for t in range(HT):
                        ps = ps_tp.tile([P, P], BF16, tag="ps_t")
                        nc.tensor.transpose(
                            ps[:, :rows[kt]],
                            key_bf[:rows[kt], kt, t * P:(t + 1) * P],
                            ident_bf[:rows[kt], :rows[kt]])
                        nc.scalar.copy(keyT_bf[:, t, kt * P:kt * P + rows[kt]],
                                       ps[:, :rows[kt]])

                kproj = kv.tile([P, HT, kc], F32, tag="kproj")
                for to in range(HT):
                    ps = ps_miscp.tile([P, 512], F32, tag="ps_misc")
                    for ti in range(HT):
                        nc.tensor.matmul(ps[:, :kc],
                                         lhsT=wk_bf[:, ti, to * P:(to + 1) * P],
                                         rhs=keyT_bf[:, ti, :kc],
                                         start=(ti == 0), stop=(ti == HT - 1))
                    nc.vector.tensor_copy(kproj[:, to, :kc], ps[:, :kc])

                val_bf = kv.tile([P, nkt, H], BF16, tag="val")
                for kt in range(nkt):
                    r0 = kt * P
                    nc.gpsimd.dma_start(out=val_bf[:rows[kt], kt, :],
                                        in_=v_d[i][r0:r0 + rows[kt], :])
                mask_rep = kv.tile([QC, kc], F32, tag="mask")
                nc.gpsimd.dma_start(
                    out=mask_rep[:, :kc],
                    in_=bass.AP(tensor=m_d[i].tensor, offset=m_d[i].offset,
                                ap=[[0, QC], [1, kc]]))

                # features: feat_t[p, j, k] = tanh(kproj[p,t,k] + qproj[p,t,32i+j])
                # one broadcast tensor_tensor add per (batch, h-tile): DVE runs
                # this at 1x (step-0 dims block the packed-read modes) but
                # per-row tensor_scalar's ~250ns/row fixed cost is worse; the
                # small chunks go to the otherwise-idle GpSimd
                feats = []
                for t in range(HT):
                    ft = featp.tile([P, QC, kc], BF16, tag="feat")
                    kap = kproj[:, t, :kc]
                    in0 = bass.AP(tensor=kap.tensor, offset=kap.offset,
                                  ap=[kap.ap[0], [0, QC], [1, kc]])
                    qap = qproj[:, t, QC * i:QC * (i + 1)]
                    in1 = bass.AP(tensor=qap.tensor, offset=qap.offset,
                                  ap=[qap.ap[0], [1, QC], [0, kc]])
                    eng = nc.gpsimd if kc <= 64 else nc.vector
                    eng.tensor_add(ft[:, :, :kc], in0, in1)
                    nc.scalar.activation(out=ft[:, :, :kc], in_=ft[:, :, :kc],
                                         func=AF.Tanh)
                    feats.append(ft)

                # scores: batched M<=4 x N<=512 matmuls; rpm rows pack per
                # matmul as concatenated [rpm*kc] output on one PSUM partition,
                # wvZ stacks 4 row-packs on partitions 32g..32g+3. t outer so
                # matmuls chase the tanh tiles; g inner so consecutive matmuls
                # hit different PE col-groups and stream concurrently.
                rpm = min(16, 512 // kc)
                ngroups = math.ceil(QC / (4 * rpm))
                ps_sc = ps_scp.tile([P, 2, 512], F32, tag="ps_sc")
                scores_tmp = sm.tile([P, 2, 512], F32, tag="sctmp")
                scores = sm.tile([QC, kc], F32, tag="scores")
                for t in range(HT):
                    for r in range(4):
                        for g in range(ngroups):
                            j0 = g * 4 * rpm
                            rmax = min(4, math.ceil((QC - j0) / rpm))
                            if r >= rmax:
                                continue
                            ps = ps_sc[32 * g:32 * g + rmax, g % 2, :rpm * kc]
                            nc.tensor.matmul(
                                ps, lhsT=wvZ[:, t, r, :rmax],
                                rhs=feats[t][:, j0 + r * rpm:j0 + (r + 1) * rpm, :kc],
                                start=(t == 0 and r == 0),
                                stop=(t == HT - 1 and r == rmax - 1),
                                tile_position=(0, 32 * g))
                for g in range(ngroups):
                    j0 = g * 4 * rpm
                    rmax = min(4, math.ceil((QC - j0) / rpm))
                    nc.vector.tensor_copy(
                        scores_tmp[32 * g:32 * g + rmax, g % 2, :rpm * kc],
                        ps_sc[32 * g:32 * g + rmax, g % 2, :rpm * kc])
                for g in range(ngroups):
                    j0 = g * 4 * rpm
                    rmax = min(4, math.ceil((QC - j0) / rpm))
                    src = scores_tmp[32 * g:32 * g + rmax, g % 2, :rpm * kc]
                    src = bass.AP(tensor=src.tensor, offset=src.offset,
                                  ap=[src.ap[0], [kc, rpm], [1, kc]])
                    nc.scalar.dma_start(out=scores[j0:j0 + rmax * rpm, :kc],
                                        in_=src)

                nc.vector.tensor_add(scores[:, :kc], scores[:, :kc],
                                     mask_rep[:, :kc])

                negmax = sm.tile([QC, 1], F32, tag="negmax")
                nc.vector.reduce_max(out=negmax, in_=scores[:, :kc],
                                     axis=mybir.AxisListType.X, negate=True)
                probs = sm.tile([QC, kc], F32, tag="probs")
                sumexp = sm.tile([QC, 1], F32, tag="sumexp")
                nc.scalar.activation(out=probs[:, :kc], in_=scores[:, :kc],
                                     func=AF.Exp, bias=negmax,
                                     accum_out=sumexp)
                rsum = sm.tile([QC, 1], F32, tag="rsum")
                nc.vector.reciprocal(rsum, sumexp)
                probs_bf = sm.tile([QC, kc], BF16, tag="probsb")
                nc.vector.tensor_scalar_mul(probs_bf[:, :kc], probs[:, :kc], rsum)

                attnT = sm.tile([P, nkt, QC], BF16, tag="attnT")
                for kt in range(nkt):
                    ps_t = ps_tp.tile([P, P], BF16, tag="ps_t")
                    nc.tensor.transpose(ps_t[:rows[kt], :QC],
                                        probs_bf[:, kt * P:kt * P + rows[kt]],
                                        ident_bf[:QC, :QC])
                    nc.vector.tensor_copy(attnT[:rows[kt], kt, :],
                                          ps_t[:rows[kt], :QC])

                ps_c = ps_ctxp.tile([P, 512], F32, tag="ps_c")
                for kt in range(nkt):
                    nc.tensor.matmul(ps_c[:QC, :],
                                     lhsT=attnT[:rows[kt], kt, :],
                                     rhs=val_bf[:rows[kt], kt, :],
                                     start=(kt == 0), stop=(kt == nkt - 1))
                ctx = sm.tile([QC, H], F32, tag="ctx")
                nc.vector.tensor_copy(ctx, ps_c[:QC, :])
                nc.scalar.dma_start(out=out_d[i], in_=ctx)

    nc.compile()
    return nc


_CACHE: dict = {}
LAST_RESULT = None


def _get_program(kcs: tuple[int, ...]):
    if kcs not in _CACHE:
        _CACHE[kcs] = _build(kcs)
    return _CACHE[kcs]


def kernel(query, key, value, valid_lens, Wq, Wk, wv):
    query = np.ascontiguousarray(np.asarray(query, dtype=np.float32))
    key = np.ascontiguousarray(np.asarray(key, dtype=np.float32))
    value = np.ascontiguousarray(np.asarray(value, dtype=np.float32))
    Wq = np.ascontiguousarray(np.asarray(Wq, dtype=np.float32))
    Wk = np.ascontiguousarray(np.asarray(Wk, dtype=np.float32))
    wv = np.ascontiguousarray(np.asarray(wv, dtype=np.float32))
    vl = np.asarray(valid_lens).astype(np.int64)

    out = np.empty((B, Q, H), dtype=np.float32)

    # fully-masked batches: reference softmax of an all -1e9 row is uniform
    active = [b for b in range(B) if vl[b] > 0]
    for b in range(B):
        if vl[b] <= 0:
            out[b, :, :] = value[b].mean(axis=0)[None, :]

    if not active:
        return out

    # sort big k-extent first so the heavy chunks schedule early
    kcs_b = {b: min(K, int(math.ceil(vl[b] / 32)) * 32) for b in active}
    order = sorted(active, key=lambda b: -kcs_b[b])
    kcs = tuple(kcs_b[b] for b in order)

    nc = _get_program(kcs)

    shared = {"wq": Wq, "wk": Wk, "wv": wv}
    for i, b in enumerate(order):
        kc = kcs[i]
        shared[f"k{i}"] = np.ascontiguousarray(key[b, :kc, :])
        shared[f"v{i}"] = np.ascontiguousarray(value[b, :kc, :])
        m = np.zeros(kc, dtype=np.float32)
        m[min(vl[b], kc):] = NEG
        shared[f"m{i}"] = m

    in_maps = []
    for ci in range(N_CORES):
        q_rows = np.ascontiguousarray(
            np.stack([query[b, QC * ci:QC * (ci + 1), :] for b in order])
        ).reshape(len(order) * QC, H)
        in_maps.append({**shared, "q": q_rows})

    res = run_bass_kernel_spmd(nc, in_maps, core_ids=list(range(N_CORES)))
    global LAST_RESULT
    LAST_RESULT = res

    for ci in range(N_CORES):
        o = res.results[ci]["out"]
        for i, b in enumerate(order):
            out[b, QC * ci:QC * (ci + 1), :] = o[i]
    return out


# revision 25
# speedup vs baseline: 1.0830x; 1.0830x over previous
"""AdditiveAttention Trainium2 kernel (8 NeuronCores, SPMD data-parallel).

reference:
    q = query @ Wq; k = key @ Wk
    scores[b,q,k] = sum_h wv[h] * tanh(q[b,q,h] + k[b,k,h])   (masked k >= valid_len)
    out = softmax(scores) @ value

Sharding: core i takes q-rows [32i, 32i+32) of EVERY batch, so all 8 cores run an
identical instruction stream (chunk per batch with that batch's k-extent) and the
per-batch valid_len-dependent work (k <= Kc_b, Kc_b = valid_len rounded up to 32)
is split exactly evenly.

Per-core dataflow (per batch chunk; partition dim = hidden):
    DMA-cast q/key/Wq/Wk to bf16, DMA-transpose -> hT layouts
    PE: qT' = Wq^T qT, kT' = Wk^T kT  (bf16)
    DVE: per q-row tensor_scalar_add (kT' + q-col) -> feat tile [128h, 32q*Kc]
    ACT: tanh in-place (bf16), one instruction per (batch, h-tile)
    PE:  scores row = wv^T @ feat[:, j, :]  M=1 matmuls, col-tiled 4-wide so 4
         q-rows land on PSUM partitions {0,32,64,96} -> 4-lane DVE copies
    softmax: reduce_max(negate) -> exp(bias=-max, accum_out=sum) -> reciprocal -> scale
    PE: transpose attn, context = attnT^T @ value (f32)
"""

import contextlib
import ctypes
import math
import sys
import types

if "/opt/trn_rl_repo" not in sys.path:
    sys.path.insert(0, "/opt/trn_rl_repo")

import numpy as np


def _install_ntff_hook():
    """This image's antenv package lacks axon_hooks; inject an equivalent so
    run_bass_kernel_spmd can trace (BASS_TRACE=1) instead of crashing."""
    if "antenv.axon_hooks" in sys.modules:
        return
    mod = types.ModuleType("antenv.axon_hooks")
    _state = {"hook": None}
    mod.set_axon_ntff_profile_hook = lambda h: _state.__setitem__("hook", h)
    mod.get_axon_ntff_profile_hook = lambda: _state["hook"]
    try:
        import antenv

        antenv.axon_hooks = mod
    except ImportError:
        pass
    sys.modules["antenv.axon_hooks"] = mod

    try:
        lib = ctypes.CDLL("/opt/axon/libaxon_pjrt.so")
    except OSError:
        return
    if not hasattr(lib, "axon_start_nrt_profile"):
        return
    lib.axon_start_nrt_profile.argtypes = [ctypes.POINTER(ctypes.c_int64),
                                           ctypes.c_size_t]
    lib.axon_start_nrt_profile.restype = ctypes.c_int64
    lib.axon_stop_nrt_profile.argtypes = [ctypes.c_char_p]
    lib.axon_stop_nrt_profile.restype = ctypes.c_int64

    @contextlib.contextmanager
    def _hook(output_dir, device_ids):
        import jax

        jax.devices()
        if device_ids:
            ids = (ctypes.c_int64 * len(device_ids))(*device_ids)
            rc = lib.axon_start_nrt_profile(ids, len(device_ids))
        else:
            rc = lib.axon_start_nrt_profile(None, 0)
        if rc != 0:
            raise RuntimeError(f"axon_start_nrt_profile rc={rc}")
        try:
            yield
        finally:
            n = lib.axon_stop_nrt_profile(str(output_dir).encode())
            print(f"profile: {n} file(s) written to {output_dir}", file=sys.stderr)

    mod.set_axon_ntff_profile_hook(_hook)


_install_ntff_hook()

import concourse.bass as bass
import concourse.bacc as bacc
import concourse.tile as tile
from concourse import mybir
from concourse.bass_utils import run_bass_kernel_spmd
from concourse.masks import make_identity

F32 = mybir.dt.float32
BF16 = mybir.dt.bfloat16
AF = mybir.ActivationFunctionType

B, Q, K, H = 4, 256, 512 // 2, 512
P = 128
HT = H // P  # 4 h-tiles
N_CORES = 8
QC = Q // N_CORES  # 32 q rows per (core, batch)
NEG = -1e9


def _build(kcs: tuple[int, ...]):
    """Build + compile the SPMD program for the given per-chunk k extents
    (one chunk per active batch, every extent a multiple of 32, <= 256)."""
    nb = len(kcs)
    R = nb * QC  # total q rows per core

    nc = bacc.Bacc("TRN2", target_bir_lowering=False, debug=False,
                   num_devices=N_CORES)

    q_d = nc.dram_tensor("q", [R, H], F32, kind="ExternalInput").ap()
    wq_d = nc.dram_tensor("wq", [H, H], F32, kind="ExternalInput").ap()
    wk_d = nc.dram_tensor("wk", [H, H], F32, kind="ExternalInput").ap()
    wv_d = nc.dram_tensor("wv", [H], F32, kind="ExternalInput").ap()
    k_d, v_d, m_d = [], [], []
    for i, kc in enumerate(kcs):
        k_d.append(nc.dram_tensor(f"k{i}", [kc, H], F32, kind="ExternalInput").ap())
        v_d.append(nc.dram_tensor(f"v{i}", [kc, H], F32, kind="ExternalInput").ap())
        m_d.append(nc.dram_tensor(f"m{i}", [kc], F32, kind="ExternalInput").ap())
    out_d = nc.dram_tensor("out", [nb, QC, H], F32, kind="ExternalOutput").ap()

    kcmax = max(kcs)

    with tile.TileContext(nc) as tc:
        with (
            tc.tile_pool(name="consts", bufs=1) as consts,
            tc.tile_pool(name="kv", bufs=4) as kv,
            tc.tile_pool(name="feat", bufs=6) as featp,
            tc.tile_pool(name="sm", bufs=2) as sm,
            tc.tile_pool(name="ps_sc", bufs=1, space="PSUM") as ps_scp,
            tc.tile_pool(name="ps_misc", bufs=2, space="PSUM") as ps_miscp,
            tc.tile_pool(name="ps_t", bufs=2, space="PSUM") as ps_tp,
            tc.tile_pool(name="ps_ctx", bufs=2, space="PSUM") as ps_ctxp,
        ):
            # query rows first: everything upstream of the feature adds hangs
            # off the q/key casts, so they lead the SWDGE queue
            q_bf = consts.tile([P, H], BF16)
            nc.gpsimd.dma_start(out=q_bf[:R, :], in_=q_d)

            # weights, cast to bf16 during DMA; layout [p, t, h_out], h_in = t*128+p
            wk_bf = consts.tile([P, HT, H], BF16)
            nc.gpsimd.dma_start(out=wk_bf, in_=wk_d.rearrange("(t p) o -> p t o", p=P))
            wq_bf = consts.tile([P, HT, H], BF16)
            nc.gpsimd.dma_start(out=wq_bf, in_=wq_d.rearrange("(t p) o -> p t o", p=P))

            ident_bf = consts.tile([P, P], BF16)
            make_identity(nc, ident_bf)

            wv_f = consts.tile([P, HT], F32)
            nc.sync.dma_start(out=wv_f, in_=wv_d.rearrange("(t p) -> p t", p=P))
            wv_bf = consts.tile([P, HT], BF16)
            nc.vector.tensor_copy(wv_bf, wv_f)
            # wvZ[:, t, r, :] = wv tile t in column r, zeros elsewhere: an M<=4
            # score matmul with this lhsT adds row-pack r's scores at PSUM
            # partition base+r and zeros into the others (which accumulate away)
            wvZ = consts.tile([P, HT, 4, 4], BF16)
            nc.vector.memset(wvZ, 0.0)
            for t in range(HT):
                for r in range(4):
                    nc.vector.tensor_copy(wvZ[:, t, r, r:r + 1], wv_bf[:, t:t + 1])

            # qT via PE transpose (PE is idle early; DMA-transpose costs ~1.2us
            # apiece on the sync ring and serializes the startup)
            qT_bf = consts.tile([P, HT, R], BF16)
            for t in range(HT):
                ps = ps_tp.tile([P, P], BF16, tag="ps_t")
                nc.tensor.transpose(ps[:, :R], q_bf[:R, t * P:(t + 1) * P],
                                    ident_bf[:R, :R])
                nc.scalar.copy(qT_bf[:, t, :], ps[:, :R])

            # qproj[p, t_out, r] (f32): h_out = t_out*128+p
            qproj = consts.tile([P, HT, R], F32)
            for to in range(HT):
                ps = ps_miscp.tile([P, 512], F32, tag="ps_misc")
                for ti in range(HT):
                    nc.tensor.matmul(ps[:, :R], lhsT=wq_bf[:, ti, to * P:(to + 1) * P],
                                     rhs=qT_bf[:, ti, :],
                                     start=(ti == 0), stop=(ti == HT - 1))
                nc.vector.tensor_copy(qproj[:, to, :], ps[:, :R])

            # ---- phase 1: per-chunk input prep (keys in, transpose, project)
            # all upfront while ACT/DVE are otherwise idle, so the compute
            # phase never waits on an input chain
            keyTs, kprojs, vals, masks = [], [], [], []
            for i, kc in enumerate(kcs):
                nkt = math.ceil(kc / P)
                rows = [min(P, kc - kt * P) for kt in range(nkt)]

                key_bf = kv.tile([P, nkt, H], BF16, tag="key")
                for kt in range(nkt):
                    r0 = kt * P
                    nc.gpsimd.dma_start(out=key_bf[:rows[kt], kt, :],
                                        in_=k_d[i][r0:r0 + rows[kt], :])

                keyT_bf = kv.tile([P, HT, kc], BF16, tag="keyT")
                for kt in range(nkt):
                    for t in range(HT):
                        ps = ps_tp.tile([P, P], BF16, tag="ps_t")
                        nc.tensor.transpose(
                            ps[:, :rows[kt]],
                            key_bf[:rows[kt], kt, t * P:(t + 1) * P],
                            ident_bf[:rows[kt], :rows[kt]])
                        nc.scalar.copy(keyT_bf[:, t, kt * P:kt * P + rows[kt]],
                                       ps[:, :rows[kt]])

                kproj = kv.tile([P, HT, kc], F32, tag="kproj")
                for to in range(HT):
                    ps = ps_miscp.tile([P, 512], F32, tag="ps_misc")
                    for ti in range(HT):
                        nc.tensor.matmul(ps[:, :kc],
                                         lhsT=wk_bf[:, ti, to * P:(to + 1) * P],
                                         rhs=keyT_bf[:, ti, :kc],
                                         start=(ti == 0), stop=(ti == HT - 1))
                    nc.vector.tensor_copy(kproj[:, to, :kc], ps[:, :kc])

                val_bf = kv.tile([P, nkt, H], BF16, tag="val")
                for kt in range(nkt):
                    r0 = kt * P
                    nc.gpsimd.dma_start(out=val_bf[:rows[kt], kt, :],
                                        in_=v_d[i][r0:r0 + rows[kt], :])
                mask_rep = kv.tile([QC, kc], F32, tag="mask")
                nc.gpsimd.dma_start(
                    out=mask_rep[:, :kc],
                    in_=bass.AP(tensor=m_d[i].tensor, offset=m_d[i].offset,
                                ap=[[0, QC], [1, kc]]))
                keyTs.append(keyT_bf)
                kprojs.append(kproj)
                vals.append(val_bf)
                masks.append(mask_rep)

            # ---- phase 2: compute per chunk
            for i, kc in enumerate(kcs):
                nkt = math.ceil(kc / P)
                rows = [min(P, kc - kt * P) for kt in range(nkt)]
                kproj, val_bf, mask_rep = kprojs[i], vals[i], masks[i]

                # features: feat_t[p, j, k] = tanh(kproj[p,t,k] + qproj[p,t,32i+j])
                # one broadcast tensor_tensor add per (batch, h-tile): DVE runs
                # this at 1x (step-0 dims block the packed-read modes) but
                # per-row tensor_scalar's ~250ns/row fixed cost is worse; the
                # small chunks go to the otherwise-idle GpSimd
                feats = []
                for t in range(HT):
                    ft = featp.tile([P, QC, kc], BF16, tag="feat")
                    kap = kproj[:, t, :kc]
                    in0 = bass.AP(tensor=kap.tensor, offset=kap.offset,
                                  ap=[kap.ap[0], [0, QC], [1, kc]])
                    qap = qproj[:, t, QC * i:QC * (i + 1)]
                    in1 = bass.AP(tensor=qap.tensor, offset=qap.offset,
                                  ap=[qap.ap[0], [1, QC], [0, kc]])
                    eng = nc.gpsimd if kc <= 64 else nc.vector
                    eng.tensor_add(ft[:, :, :kc], in0, in1)
                    nc.scalar.activation(out=ft[:, :, :kc], in_=ft[:, :, :kc],
                                         func=AF.Tanh)
                    feats.append(ft)

                # scores: batched M<=4 x N<=512 matmuls; rpm rows pack per
                # matmul as concatenated [rpm*kc] output on one PSUM partition,
                # wvZ stacks 4 row-packs on partitions 32g..32g+3. t outer so
                # matmuls chase the tanh tiles; g inner so consecutive matmuls
                # hit different PE col-groups and stream concurrently.
                rpm = min(16, 512 // kc)
                ngroups = math.ceil(QC / (4 * rpm))
                ps_sc = ps_scp.tile([P, 2, 512], F32, tag="ps_sc")
                scores_tmp = sm.tile([P, 2, 512], F32, tag="sctmp")
                scores = sm.tile([QC, kc], F32, tag="scores")
                for t in range(HT):
                    for r in range(4):
                        for g in range(ngroups):
                            j0 = g * 4 * rpm
                            rmax = min(4, math.ceil((QC - j0) / rpm))
                            if r >= rmax:
                                continue
                            ps = ps_sc[32 * g:32 * g + rmax, g % 2, :rpm * kc]
                            nc.tensor.matmul(
                                ps, lhsT=wvZ[:, t, r, :rmax],
                                rhs=feats[t][:, j0 + r * rpm:j0 + (r + 1) * rpm, :kc],
                                start=(t == 0 and r == 0),
                                stop=(t == HT - 1 and r == rmax - 1),
                                tile_position=(0, 32 * g))
                for g in range(ngroups):
                    j0 = g * 4 * rpm
                    rmax = min(4, math.ceil((QC - j0) / rpm))
                    nc.vector.tensor_copy(
                        scores_tmp[32 * g:32 * g + rmax, g % 2, :rpm * kc],
                        ps_sc[32 * g:32 * g + rmax, g % 2, :rpm * kc])
                for g in range(ngroups):
                    j0 = g * 4 * rpm
                    rmax = min(4, math.ceil((QC - j0) / rpm))
                    src = scores_tmp[32 * g:32 * g + rmax, g % 2, :rpm * kc]
                    src = bass.AP(tensor=src.tensor, offset=src.offset,
                                  ap=[src.ap[0], [kc, rpm], [1, kc]])
                    nc.scalar.dma_start(out=scores[j0:j0 + rmax * rpm, :kc],
                                        in_=src)

                nc.vector.tensor_add(scores[:, :kc], scores[:, :kc],
                                     mask_rep[:, :kc])

                negmax = sm.tile([QC, 1], F32, tag="negmax")
                nc.vector.reduce_max(out=negmax, in_=scores[:, :kc],
                                     axis=mybir.AxisListType.X, negate=True)
                probs = sm.tile([QC, kc], F32, tag="probs")
                sumexp = sm.tile([QC, 1], F32, tag="sumexp")
                nc.scalar.activation(out=probs[:, :kc], in_=scores[:, :kc],
                                     func=AF.Exp, bias=negmax,
                                     accum_out=sumexp)
                rsum = sm.tile([QC, 1], F32, tag="rsum")
                nc.vector.reciprocal(rsum, sumexp)
                probs_bf = sm.tile([QC, kc], BF16, tag="probsb")
                nc.vector.tensor_scalar_mul(probs_bf[:, :kc], probs[:, :kc], rsum)

                attnT = sm.tile([P, nkt, QC], BF16, tag="attnT")
                for kt in range(nkt):
                    ps_t = ps_tp.tile([P, P], BF16, tag="ps_t")
                    nc.tensor.transpose(ps_t[:rows[kt], :QC],
                                        probs_bf[:, kt * P:kt * P + rows[kt]],
                                        ident_bf[:QC, :QC])
                    nc.vector.tensor_copy(attnT[:rows[kt], kt, :],
                                          ps_t[:rows[kt], :QC])

                ps_c = ps_ctxp.tile([P, 512], F32, tag="ps_c")
                for kt in range(nkt):
                    nc.tensor.matmul(ps_c[:QC, :],
                                     lhsT=attnT[:rows[kt], kt, :],
                                     rhs=val_bf[:rows[kt], kt, :],
                                     start=(kt == 0), stop=(kt == nkt - 1))
                ctx = sm.tile([QC, H], F32, tag="ctx")
                nc.vector.tensor_copy(ctx, ps_c[:QC, :])
                nc.scalar.dma_start(out=out_d[i], in_=ctx)

    nc.compile()
    return nc


_CACHE: dict = {}
LAST_RESULT = None


def _get_program(kcs: tuple[int, ...]):
    if kcs not in _CACHE:
        _CACHE[kcs] = _build(kcs)
    return _CACHE[kcs]


def kernel(query, key, value, valid_lens, Wq, Wk, wv):
    query = np.ascontiguousarray(np.asarray(query, dtype=np.float32))
    key = np.ascontiguousarray(np.asarray(key, dtype=np.float32))
    value = np.ascontiguousarray(np.asarray(value, dtype=np.float32))
    Wq = np.ascontiguousarray(np.asarray(Wq, dtype=np.float32))
    Wk = np.ascontiguousarray(np.asarray(Wk, dtype=np.float32))
    wv = np.ascontiguousarray(np.asarray(wv, dtype=np.float32))
    vl = np.asarray(valid_lens).astype(np.int64)

    out = np.empty((B, Q, H), dtype=np.float32)

    # fully-masked batches: reference softmax of an all -1e9 row is uniform
    active = [b for b in range(B) if vl[b] > 0]
    for b in range(B):
        if vl[b] <= 0:
            out[b, :, :] = value[b].mean(axis=0)[None, :]

    if not active:
        return out

    # sort big k-extent first so the heavy chunks schedule early
    kcs_b = {b: min(K, int(math.ceil(vl[b] / 32)) * 32) for b in active}
    order = sorted(active, key=lambda b: -kcs_b[b])
    kcs = tuple(kcs_b[b] for b in order)

    nc = _get_program(kcs)

    shared = {"wq": Wq, "wk": Wk, "wv": wv}
    for i, b in enumerate(order):
        kc = kcs[i]
        shared[f"k{i}"] = np.ascontiguousarray(key[b, :kc, :])
        shared[f"v{i}"] = np.ascontiguousarray(value[b, :kc, :])
        m = np.zeros(kc, dtype=np.float32)
        m[min(vl[b], kc):] = NEG
        shared[f"m{i}"] = m

    in_maps = []
    for ci in range(N_CORES):
        q_rows = np.ascontiguousarray(
            np.stack([query[b, QC * ci:QC * (ci + 1), :] for b in order])
        ).reshape(len(order) * QC, H)
        in_maps.append({**shared, "q": q_rows})

    res = run_bass_kernel_spmd(nc, in_maps, core_ids=list(range(N_CORES)))
    global LAST_RESULT
    LAST_RESULT = res

    for ci in range(N_CORES):
        o = res.results[ci]["out"]
        for i, b in enumerate(order):
            out[b, QC * ci:QC * (ci + 1), :] = o[i]
    return out


# revision 26
# speedup vs baseline: 1.2567x; 1.1603x over previous
"""AdditiveAttention Trainium2 kernel (8 NeuronCores, SPMD data-parallel).

reference:
    q = query @ Wq; k = key @ Wk
    scores[b,q,k] = sum_h wv[h] * tanh(q[b,q,h] + k[b,k,h])   (masked k >= valid_len)
    out = softmax(scores) @ value

Sharding: core i takes q-rows [32i, 32i+32) of EVERY batch, so all 8 cores run an
identical instruction stream (chunk per batch with that batch's k-extent) and the
per-batch valid_len-dependent work (k <= Kc_b, Kc_b = valid_len rounded up to 32)
is split exactly evenly.

Per-core dataflow (per batch chunk; partition dim = hidden):
    DMA-cast q/key/Wq/Wk to bf16, DMA-transpose -> hT layouts
    PE: qT' = Wq^T qT, kT' = Wk^T kT  (bf16)
    DVE: per q-row tensor_scalar_add (kT' + q-col) -> feat tile [128h, 32q*Kc]
    ACT: tanh in-place (bf16), one instruction per (batch, h-tile)
    PE:  scores row = wv^T @ feat[:, j, :]  M=1 matmuls, col-tiled 4-wide so 4
         q-rows land on PSUM partitions {0,32,64,96} -> 4-lane DVE copies
    softmax: reduce_max(negate) -> exp(bias=-max, accum_out=sum) -> reciprocal -> scale
    PE: transpose attn, context = attnT^T @ value (f32)
"""

import contextlib
import ctypes
import math
import sys
import types

if "/opt/trn_rl_repo" not in sys.path:
    sys.path.insert(0, "/opt/trn_rl_repo")

import numpy as np


def _install_ntff_hook():
    """This image's antenv package lacks axon_hooks; inject an equivalent so
    run_bass_kernel_spmd can trace (BASS_TRACE=1) instead of crashing."""
    if "antenv.axon_hooks" in sys.modules:
        return
    mod = types.ModuleType("antenv.axon_hooks")
    _state = {"hook": None}
    mod.set_axon_ntff_profile_hook = lambda h: _state.__setitem__("hook", h)
    mod.get_axon_ntff_profile_hook = lambda: _state["hook"]
    try:
        import antenv

        antenv.axon_hooks = mod
    except ImportError:
        pass
    sys.modules["antenv.axon_hooks"] = mod

    try:
        lib = ctypes.CDLL("/opt/axon/libaxon_pjrt.so")
    except OSError:
        return
    if not hasattr(lib, "axon_start_nrt_profile"):
        return
    lib.axon_start_nrt_profile.argtypes = [ctypes.POINTER(ctypes.c_int64),
                                           ctypes.c_size_t]
    lib.axon_start_nrt_profile.restype = ctypes.c_int64
    lib.axon_stop_nrt_profile.argtypes = [ctypes.c_char_p]
    lib.axon_stop_nrt_profile.restype = ctypes.c_int64

    @contextlib.contextmanager
    def _hook(output_dir, device_ids):
        import jax

        jax.devices()
        if device_ids:
            ids = (ctypes.c_int64 * len(device_ids))(*device_ids)
            rc = lib.axon_start_nrt_profile(ids, len(device_ids))
        else:
            rc = lib.axon_start_nrt_profile(None, 0)
        if rc != 0:
            raise RuntimeError(f"axon_start_nrt_profile rc={rc}")
        try:
            yield
        finally:
            n = lib.axon_stop_nrt_profile(str(output_dir).encode())
            print(f"profile: {n} file(s) written to {output_dir}", file=sys.stderr)

    mod.set_axon_ntff_profile_hook(_hook)


_install_ntff_hook()

import concourse.bass as bass
import concourse.bacc as bacc
import concourse.tile as tile
from concourse import mybir
from concourse.bass_utils import run_bass_kernel_spmd
from concourse.masks import make_identity

F32 = mybir.dt.float32
BF16 = mybir.dt.bfloat16
AF = mybir.ActivationFunctionType

B, Q, K, H = 4, 256, 512 // 2, 512
P = 128
HT = H // P  # 4 h-tiles
N_CORES = 8
QC = Q // N_CORES  # 32 q rows per (core, batch)
NEG = -1e9


def _build(kcs: tuple[int, ...]):
    """Build + compile the SPMD program for the given per-chunk k extents
    (one chunk per active batch, every extent a multiple of 32, <= 256)."""
    nb = len(kcs)
    R = nb * QC  # total q rows per core

    nc = bacc.Bacc("TRN2", target_bir_lowering=False, debug=False,
                   num_devices=N_CORES)

    q_d = nc.dram_tensor("q", [R, H], F32, kind="ExternalInput").ap()
    wq_d = nc.dram_tensor("wq", [H, H], F32, kind="ExternalInput").ap()
    wk_d = nc.dram_tensor("wk", [H, H], F32, kind="ExternalInput").ap()
    wv_d = nc.dram_tensor("wv", [H], F32, kind="ExternalInput").ap()
    k_d, v_d, m_d = [], [], []
    for i, kc in enumerate(kcs):
        k_d.append(nc.dram_tensor(f"k{i}", [kc, H], F32, kind="ExternalInput").ap())
        v_d.append(nc.dram_tensor(f"v{i}", [kc, H], F32, kind="ExternalInput").ap())
        m_d.append(nc.dram_tensor(f"m{i}", [kc], F32, kind="ExternalInput").ap())
    out_d = nc.dram_tensor("out", [nb, QC, H], F32, kind="ExternalOutput").ap()

    kcmax = max(kcs)

    with tile.TileContext(nc) as tc:
        with (
            tc.tile_pool(name="consts", bufs=1) as consts,
            tc.tile_pool(name="kv", bufs=4) as kv,
            tc.tile_pool(name="feat", bufs=6) as featp,
            tc.tile_pool(name="sm", bufs=2) as sm,
            tc.tile_pool(name="ps_sc", bufs=1, space="PSUM") as ps_scp,
            tc.tile_pool(name="ps_misc", bufs=2, space="PSUM") as ps_miscp,
            tc.tile_pool(name="ps_t", bufs=2, space="PSUM") as ps_tp,
            tc.tile_pool(name="ps_ctx", bufs=2, space="PSUM") as ps_ctxp,
        ):
            # query rows first: everything upstream of the feature adds hangs
            # off the q/key casts, so they lead the SWDGE queue
            q_bf = consts.tile([P, H], BF16)
            nc.gpsimd.dma_start(out=q_bf[:R, :], in_=q_d)

            # weights, cast to bf16 during DMA; layout [p, t, h_out], h_in = t*128+p
            wk_bf = consts.tile([P, HT, H], BF16)
            nc.gpsimd.dma_start(out=wk_bf, in_=wk_d.rearrange("(t p) o -> p t o", p=P))
            wq_bf = consts.tile([P, HT, H], BF16)
            nc.gpsimd.dma_start(out=wq_bf, in_=wq_d.rearrange("(t p) o -> p t o", p=P))

            ident_bf = consts.tile([P, P], BF16)
            make_identity(nc, ident_bf)

            wv_f = consts.tile([P, HT], F32)
            nc.sync.dma_start(out=wv_f, in_=wv_d.rearrange("(t p) -> p t", p=P))
            wv_bf = consts.tile([P, HT], BF16)
            nc.vector.tensor_copy(wv_bf, wv_f)
            # wvZ[:, t, r, :] = wv tile t in column r, zeros elsewhere: an M<=4
            # score matmul with this lhsT adds row-pack r's scores at PSUM
            # partition base+r and zeros into the others (which accumulate away)
            wvZ = consts.tile([P, HT, 4, 4], BF16)
            nc.vector.memset(wvZ, 0.0)
            for t in range(HT):
                for r in range(4):
                    nc.vector.tensor_copy(wvZ[:, t, r, r:r + 1], wv_bf[:, t:t + 1])

            # qT via PE transpose (PE is idle early; DMA-transpose costs ~1.2us
            # apiece on the sync ring and serializes the startup)
            qT_bf = consts.tile([P, HT, R], BF16)
            for t in range(HT):
                ps = ps_tp.tile([P, P], BF16, tag="ps_t")
                nc.tensor.transpose(ps[:, :R], q_bf[:R, t * P:(t + 1) * P],
                                    ident_bf[:R, :R])
                nc.scalar.copy(qT_bf[:, t, :], ps[:, :R])

            # qproj[p, t_out, r] (f32): h_out = t_out*128+p
            qproj = consts.tile([P, HT, R], F32)
            for to in range(HT):
                ps = ps_miscp.tile([P, 512], F32, tag="ps_misc")
                for ti in range(HT):
                    nc.tensor.matmul(ps[:, :R], lhsT=wq_bf[:, ti, to * P:(to + 1) * P],
                                     rhs=qT_bf[:, ti, :],
                                     start=(ti == 0), stop=(ti == HT - 1))
                nc.vector.tensor_copy(qproj[:, to, :], ps[:, :R])

            # ---- phase 0: all remaining cast-DMAs (SWDGE) before DVE gets
            # busy -- descriptor-ring writes contend with DVE's SBUF ports
            keys, vals, masks = [], [], []
            for i, kc in enumerate(kcs):
                nkt = math.ceil(kc / P)
                rows = [min(P, kc - kt * P) for kt in range(nkt)]
                key_bf = kv.tile([P, nkt, H], BF16, tag="key")
                for kt in range(nkt):
                    r0 = kt * P
                    nc.gpsimd.dma_start(out=key_bf[:rows[kt], kt, :],
                                        in_=k_d[i][r0:r0 + rows[kt], :])
                keys.append(key_bf)
            for i, kc in enumerate(kcs):
                nkt = math.ceil(kc / P)
                rows = [min(P, kc - kt * P) for kt in range(nkt)]
                val_bf = kv.tile([P, nkt, H], BF16, tag="val")
                for kt in range(nkt):
                    r0 = kt * P
                    nc.gpsimd.dma_start(out=val_bf[:rows[kt], kt, :],
                                        in_=v_d[i][r0:r0 + rows[kt], :])
                mask_rep = kv.tile([QC, kc], F32, tag="mask")
                nc.gpsimd.dma_start(
                    out=mask_rep[:, :kc],
                    in_=bass.AP(tensor=m_d[i].tensor, offset=m_d[i].offset,
                                ap=[[0, QC], [1, kc]]))
                vals.append(val_bf)
                masks.append(mask_rep)

            # ---- phase 1: per-chunk transpose + key projection
            keyTs, kprojs = [], []
            for i, kc in enumerate(kcs):
                nkt = math.ceil(kc / P)
                rows = [min(P, kc - kt * P) for kt in range(nkt)]
                key_bf = keys[i]

                keyT_bf = kv.tile([P, HT, kc], BF16, tag="keyT")
                for kt in range(nkt):
                    for t in range(HT):
                        ps = ps_tp.tile([P, P], BF16, tag="ps_t")
                        nc.tensor.transpose(
                            ps[:, :rows[kt]],
                            key_bf[:rows[kt], kt, t * P:(t + 1) * P],
                            ident_bf[:rows[kt], :rows[kt]])
                        nc.scalar.copy(keyT_bf[:, t, kt * P:kt * P + rows[kt]],
                                       ps[:, :rows[kt]])

                kproj = kv.tile([P, HT, kc], F32, tag="kproj")
                for to in range(HT):
                    ps = ps_miscp.tile([P, 512], F32, tag="ps_misc")
                    for ti in range(HT):
                        nc.tensor.matmul(ps[:, :kc],
                                         lhsT=wk_bf[:, ti, to * P:(to + 1) * P],
                                         rhs=keyT_bf[:, ti, :kc],
                                         start=(ti == 0), stop=(ti == HT - 1))
                    nc.vector.tensor_copy(kproj[:, to, :kc], ps[:, :kc])

                keyTs.append(keyT_bf)
                kprojs.append(kproj)

            # ---- phase 2: compute per chunk
            for i, kc in enumerate(kcs):
                nkt = math.ceil(kc / P)
                rows = [min(P, kc - kt * P) for kt in range(nkt)]
                kproj, val_bf, mask_rep = kprojs[i], vals[i], masks[i]

                # features: feat_t[p, j, k] = tanh(kproj[p,t,k] + qproj[p,t,32i+j])
                # one broadcast tensor_tensor add per (batch, h-tile): DVE runs
                # this at 1x (step-0 dims block the packed-read modes) but
                # per-row tensor_scalar's ~250ns/row fixed cost is worse; the
                # small chunks go to the otherwise-idle GpSimd
                feats = []
                for t in range(HT):
                    ft = featp.tile([P, QC, kc], BF16, tag="feat")
                    kap = kproj[:, t, :kc]
                    in0 = bass.AP(tensor=kap.tensor, offset=kap.offset,
                                  ap=[kap.ap[0], [0, QC], [1, kc]])
                    qap = qproj[:, t, QC * i:QC * (i + 1)]
                    in1 = bass.AP(tensor=qap.tensor, offset=qap.offset,
                                  ap=[qap.ap[0], [1, QC], [0, kc]])
                    eng = nc.gpsimd if (kc <= 64 or (kc == 224 and t < 2)) \
                        else nc.vector
                    eng.tensor_add(ft[:, :, :kc], in0, in1)
                    nc.scalar.activation(out=ft[:, :, :kc], in_=ft[:, :, :kc],
                                         func=AF.Tanh)
                    feats.append(ft)

                # scores: batched M<=4 x N<=512 matmuls; rpm rows pack per
                # matmul as concatenated [rpm*kc] output on one PSUM partition,
                # wvZ stacks 4 row-packs on partitions 32g..32g+3. t outer so
                # matmuls chase the tanh tiles; g inner so consecutive matmuls
                # hit different PE col-groups and stream concurrently.
                rpm = min(16, 512 // kc)
                ngroups = math.ceil(QC / (4 * rpm))
                ps_sc = ps_scp.tile([P, 2, 512], F32, tag="ps_sc")
                scores_tmp = sm.tile([P, 2, 512], F32, tag="sctmp")
                scores = sm.tile([QC, kc], F32, tag="scores")
                for t in range(HT):
                    for r in range(4):
                        for g in range(ngroups):
                            j0 = g * 4 * rpm
                            rmax = min(4, math.ceil((QC - j0) / rpm))
                            if r >= rmax:
                                continue
                            ps = ps_sc[32 * g:32 * g + rmax, g % 2, :rpm * kc]
                            nc.tensor.matmul(
                                ps, lhsT=wvZ[:, t, r, :rmax],
                                rhs=feats[t][:, j0 + r * rpm:j0 + (r + 1) * rpm, :kc],
                                start=(t == 0 and r == 0),
                                stop=(t == HT - 1 and r == rmax - 1),
                                tile_position=(0, 32 * g))
                for g in range(ngroups):
                    j0 = g * 4 * rpm
                    rmax = min(4, math.ceil((QC - j0) / rpm))
                    nc.vector.tensor_copy(
                        scores_tmp[32 * g:32 * g + rmax, g % 2, :rpm * kc],
                        ps_sc[32 * g:32 * g + rmax, g % 2, :rpm * kc])
                for g in range(ngroups):
                    j0 = g * 4 * rpm
                    rmax = min(4, math.ceil((QC - j0) / rpm))
                    src = scores_tmp[32 * g:32 * g + rmax, g % 2, :rpm * kc]
                    src = bass.AP(tensor=src.tensor, offset=src.offset,
                                  ap=[src.ap[0], [kc, rpm], [1, kc]])
                    nc.sync.dma_start(out=scores[j0:j0 + rmax * rpm, :kc],
                                       in_=src)

                nc.vector.tensor_add(scores[:, :kc], scores[:, :kc],
                                     mask_rep[:, :kc])

                negmax = sm.tile([QC, 1], F32, tag="negmax")
                nc.vector.reduce_max(out=negmax, in_=scores[:, :kc],
                                     axis=mybir.AxisListType.X, negate=True)
                probs = sm.tile([QC, kc], F32, tag="probs")
                sumexp = sm.tile([QC, 1], F32, tag="sumexp")
                nc.scalar.activation(out=probs[:, :kc], in_=scores[:, :kc],
                                     func=AF.Exp, bias=negmax,
                                     accum_out=sumexp)
                rsum = sm.tile([QC, 1], F32, tag="rsum")
                nc.vector.reciprocal(rsum, sumexp)
                probs_bf = sm.tile([QC, kc], BF16, tag="probsb")
                nc.vector.tensor_scalar_mul(probs_bf[:, :kc], probs[:, :kc], rsum)

                attnT = sm.tile([P, nkt, QC], BF16, tag="attnT")
                for kt in range(nkt):
                    ps_t = ps_tp.tile([P, P], BF16, tag="ps_t")
                    nc.tensor.transpose(ps_t[:rows[kt], :QC],
                                        probs_bf[:, kt * P:kt * P + rows[kt]],
                                        ident_bf[:QC, :QC])
                    nc.vector.tensor_copy(attnT[:rows[kt], kt, :],
                                          ps_t[:rows[kt], :QC])

                ps_c = ps_ctxp.tile([P, 512], F32, tag="ps_c")
                for kt in range(nkt):
                    nc.tensor.matmul(ps_c[:QC, :],
                                     lhsT=attnT[:rows[kt], kt, :],
                                     rhs=val_bf[:rows[kt], kt, :],
                                     start=(kt == 0), stop=(kt == nkt - 1))
                ctx = sm.tile([QC, H], F32, tag="ctx")
                nc.vector.tensor_copy(ctx, ps_c[:QC, :])
                nc.sync.dma_start(out=out_d[i], in_=ctx)

    nc.compile()
    return nc


_CACHE: dict = {}
LAST_RESULT = None


def _get_program(kcs: tuple[int, ...]):
    if kcs not in _CACHE:
        _CACHE[kcs] = _build(kcs)
    return _CACHE[kcs]


def kernel(query, key, value, valid_lens, Wq, Wk, wv):
    query = np.ascontiguousarray(np.asarray(query, dtype=np.float32))
    key = np.ascontiguousarray(np.asarray(key, dtype=np.float32))
    value = np.ascontiguousarray(np.asarray(value, dtype=np.float32))
    Wq = np.ascontiguousarray(np.asarray(Wq, dtype=np.float32))
    Wk = np.ascontiguousarray(np.asarray(Wk, dtype=np.float32))
    wv = np.ascontiguousarray(np.asarray(wv, dtype=np.float32))
    vl = np.asarray(valid_lens).astype(np.int64)

    out = np.empty((B, Q, H), dtype=np.float32)

    # fully-masked batches: reference softmax of an all -1e9 row is uniform
    active = [b for b in range(B) if vl[b] > 0]
    for b in range(B):
        if vl[b] <= 0:
            out[b, :, :] = value[b].mean(axis=0)[None, :]

    if not active:
        return out

    # sort big k-extent first so the heavy chunks schedule early
    kcs_b = {b: min(K, int(math.ceil(vl[b] / 32)) * 32) for b in active}
    order = sorted(active, key=lambda b: -kcs_b[b])
    kcs = tuple(kcs_b[b] for b in order)

    nc = _get_program(kcs)

    shared = {"wq": Wq, "wk": Wk, "wv": wv}
    for i, b in enumerate(order):
        kc = kcs[i]
        shared[f"k{i}"] = np.ascontiguousarray(key[b, :kc, :])
        shared[f"v{i}"] = np.ascontiguousarray(value[b, :kc, :])
        m = np.zeros(kc, dtype=np.float32)
        m[min(vl[b], kc):] = NEG
        shared[f"m{i}"] = m

    in_maps = []
    for ci in range(N_CORES):
        q_rows = np.ascontiguousarray(
            np.stack([query[b, QC * ci:QC * (ci + 1), :] for b in order])
        ).reshape(len(order) * QC, H)
        in_maps.append({**shared, "q": q_rows})

    res = run_bass_kernel_spmd(nc, in_maps, core_ids=list(range(N_CORES)))
    global LAST_RESULT
    LAST_RESULT = res

    for ci in range(N_CORES):
        o = res.results[ci]["out"]
        for i, b in enumerate(order):
            out[b, QC * ci:QC * (ci + 1), :] = o[i]
    return out


# revision 27
# speedup vs baseline: 1.4951x; 1.1897x over previous
"""AdditiveAttention Trainium2 kernel (8 NeuronCores, SPMD data-parallel).

reference:
    q = query @ Wq; k = key @ Wk
    scores[b,q,k] = sum_h wv[h] * tanh(q[b,q,h] + k[b,k,h])   (masked k >= valid_len)
    out = softmax(scores) @ value

Sharding: core i takes q-rows [32i, 32i+32) of EVERY batch, so all 8 cores run an
identical instruction stream (chunk per batch with that batch's k-extent) and the
per-batch valid_len-dependent work (k <= Kc_b, Kc_b = valid_len rounded up to 32)
is split exactly evenly.

Per-core dataflow (per batch chunk; partition dim = hidden):
    DMA-cast q/key/Wq/Wk to bf16, DMA-transpose -> hT layouts
    PE: qT' = Wq^T qT, kT' = Wk^T kT  (bf16)
    DVE: per q-row tensor_scalar_add (kT' + q-col) -> feat tile [128h, 32q*Kc]
    ACT: tanh in-place (bf16), one instruction per (batch, h-tile)
    PE:  scores row = wv^T @ feat[:, j, :]  M=1 matmuls, col-tiled 4-wide so 4
         q-rows land on PSUM partitions {0,32,64,96} -> 4-lane DVE copies
    softmax: reduce_max(negate) -> exp(bias=-max, accum_out=sum) -> reciprocal -> scale
    PE: transpose attn, context = attnT^T @ value (f32)
"""

import contextlib
import ctypes
import math
import sys
import types

if "/opt/trn_rl_repo" not in sys.path:
    sys.path.insert(0, "/opt/trn_rl_repo")

import numpy as np


def _install_ntff_hook():
    """This image's antenv package lacks axon_hooks; inject an equivalent so
    run_bass_kernel_spmd can trace (BASS_TRACE=1) instead of crashing."""
    if "antenv.axon_hooks" in sys.modules:
        return
    mod = types.ModuleType("antenv.axon_hooks")
    _state = {"hook": None}
    mod.set_axon_ntff_profile_hook = lambda h: _state.__setitem__("hook", h)
    mod.get_axon_ntff_profile_hook = lambda: _state["hook"]
    try:
        import antenv

        antenv.axon_hooks = mod
    except ImportError:
        pass
    sys.modules["antenv.axon_hooks"] = mod

    try:
        lib = ctypes.CDLL("/opt/axon/libaxon_pjrt.so")
    except OSError:
        return
    if not hasattr(lib, "axon_start_nrt_profile"):
        return
    lib.axon_start_nrt_profile.argtypes = [ctypes.POINTER(ctypes.c_int64),
                                           ctypes.c_size_t]
    lib.axon_start_nrt_profile.restype = ctypes.c_int64
    lib.axon_stop_nrt_profile.argtypes = [ctypes.c_char_p]
    lib.axon_stop_nrt_profile.restype = ctypes.c_int64

    @contextlib.contextmanager
    def _hook(output_dir, device_ids):
        import jax

        jax.devices()
        if device_ids:
            ids = (ctypes.c_int64 * len(device_ids))(*device_ids)
            rc = lib.axon_start_nrt_profile(ids, len(device_ids))
        else:
            rc = lib.axon_start_nrt_profile(None, 0)
        if rc != 0:
            raise RuntimeError(f"axon_start_nrt_profile rc={rc}")
        try:
            yield
        finally:
            n = lib.axon_stop_nrt_profile(str(output_dir).encode())
            print(f"profile: {n} file(s) written to {output_dir}", file=sys.stderr)

    mod.set_axon_ntff_profile_hook(_hook)


_install_ntff_hook()

import concourse.bass as bass
import concourse.bacc as bacc
import concourse.tile as tile
from concourse import mybir
from concourse.bass_utils import run_bass_kernel_spmd
from concourse.masks import make_identity

F32 = mybir.dt.float32
BF16 = mybir.dt.bfloat16
AF = mybir.ActivationFunctionType

B, Q, K, H = 4, 256, 512 // 2, 512
P = 128
HT = H // P  # 4 h-tiles
N_CORES = 8
QC = Q // N_CORES  # 32 q rows per (core, batch)
NEG = -1e9


def _build(kcs: tuple[int, ...]):
    """Build + compile the SPMD program for the given per-chunk k extents
    (one chunk per active batch, every extent a multiple of 32, <= 256)."""
    nb = len(kcs)
    R = nb * QC  # total q rows per core

    nc = bacc.Bacc("TRN2", target_bir_lowering=False, debug=False,
                   num_devices=N_CORES)

    q_d = nc.dram_tensor("q", [R, H], F32, kind="ExternalInput").ap()
    wq_d = nc.dram_tensor("wq", [H, H], F32, kind="ExternalInput").ap()
    wk_d = nc.dram_tensor("wk", [H, H], F32, kind="ExternalInput").ap()
    wv_d = nc.dram_tensor("wv", [H], F32, kind="ExternalInput").ap()
    k_d, v_d, m_d = [], [], []
    for i, kc in enumerate(kcs):
        k_d.append(nc.dram_tensor(f"k{i}", [kc, H], F32, kind="ExternalInput").ap())
        v_d.append(nc.dram_tensor(f"v{i}", [kc, H], F32, kind="ExternalInput").ap())
        m_d.append(nc.dram_tensor(f"m{i}", [kc], F32, kind="ExternalInput").ap())
    out_d = nc.dram_tensor("out", [nb, QC, H], F32, kind="ExternalOutput").ap()

    kcmax = max(kcs)

    with tile.TileContext(nc) as tc:
        with (
            tc.tile_pool(name="consts", bufs=1) as consts,
            tc.tile_pool(name="kv", bufs=4) as kv,
            tc.tile_pool(name="feat", bufs=6) as featp,
            tc.tile_pool(name="sm", bufs=2) as sm,
            tc.tile_pool(name="ps_sc", bufs=1, space="PSUM") as ps_scp,
            tc.tile_pool(name="ps_misc", bufs=2, space="PSUM") as ps_miscp,
            tc.tile_pool(name="ps_t", bufs=2, space="PSUM") as ps_tp,
            tc.tile_pool(name="ps_ctx", bufs=2, space="PSUM") as ps_ctxp,
        ):
            # query rows first: everything upstream of the feature adds hangs
            # off the q/key casts, so they lead the SWDGE queue
            q_bf = consts.tile([P, H], BF16)
            nc.gpsimd.dma_start(out=q_bf[:R, :], in_=q_d)

            # weights, cast to bf16 during DMA; layout [p, t, h_out], h_in = t*128+p
            wk_bf = consts.tile([P, HT, H], BF16)
            nc.gpsimd.dma_start(out=wk_bf, in_=wk_d.rearrange("(t p) o -> p t o", p=P))
            wq_bf = consts.tile([P, HT, H], BF16)
            nc.gpsimd.dma_start(out=wq_bf, in_=wq_d.rearrange("(t p) o -> p t o", p=P))

            ident_bf = consts.tile([P, P], BF16)
            make_identity(nc, ident_bf)

            wv_f = consts.tile([P, HT], F32)
            nc.sync.dma_start(out=wv_f, in_=wv_d.rearrange("(t p) -> p t", p=P))
            wv_bf = consts.tile([P, HT], BF16)
            nc.vector.tensor_copy(wv_bf, wv_f)
            # wvZ[:, t, r, :] = wv tile t in column r, zeros elsewhere: an M<=4
            # score matmul with this lhsT adds row-pack r's scores at PSUM
            # partition base+r and zeros into the others (which accumulate away)
            wvZ = consts.tile([P, HT, 4, 4], BF16)
            nc.vector.memset(wvZ, 0.0)
            for t in range(HT):
                for r in range(4):
                    nc.vector.tensor_copy(wvZ[:, t, r, r:r + 1], wv_bf[:, t:t + 1])

            # qT via PE transpose (PE is idle early; DMA-transpose costs ~1.2us
            # apiece on the sync ring and serializes the startup)
            qT_bf = consts.tile([P, HT, R], BF16)
            for t in range(HT):
                ps = ps_tp.tile([P, P], BF16, tag="ps_t")
                nc.tensor.transpose(ps[:, :R], q_bf[:R, t * P:(t + 1) * P],
                                    ident_bf[:R, :R])
                nc.scalar.copy(qT_bf[:, t, :], ps[:, :R])

            # qproj[p, t_out, r] (f32): h_out = t_out*128+p
            qproj = consts.tile([P, HT, R], F32)
            for to in range(HT):
                ps = ps_miscp.tile([P, 512], F32, tag="ps_misc")
                for ti in range(HT):
                    nc.tensor.matmul(ps[:, :R], lhsT=wq_bf[:, ti, to * P:(to + 1) * P],
                                     rhs=qT_bf[:, ti, :],
                                     start=(ti == 0), stop=(ti == HT - 1))
                nc.vector.tensor_copy(qproj[:, to, :], ps[:, :R])

            # ---- phase 0: all remaining cast-DMAs (SWDGE) before DVE gets
            # busy -- descriptor-ring writes contend with DVE's SBUF ports
            keys, vals, masks = [], [], []
            for i, kc in enumerate(kcs):
                nkt = math.ceil(kc / P)
                rows = [min(P, kc - kt * P) for kt in range(nkt)]
                key_bf = kv.tile([P, nkt, H], BF16, tag="key")
                for kt in range(nkt):
                    r0 = kt * P
                    nc.gpsimd.dma_start(out=key_bf[:rows[kt], kt, :],
                                        in_=k_d[i][r0:r0 + rows[kt], :])
                keys.append(key_bf)
            for i, kc in enumerate(kcs):
                nkt = math.ceil(kc / P)
                rows = [min(P, kc - kt * P) for kt in range(nkt)]
                val_bf = kv.tile([P, nkt, H], BF16, tag="val")
                for kt in range(nkt):
                    r0 = kt * P
                    nc.gpsimd.dma_start(out=val_bf[:rows[kt], kt, :],
                                        in_=v_d[i][r0:r0 + rows[kt], :])
                mask_rep = kv.tile([QC, kc], F32, tag="mask")
                nc.gpsimd.dma_start(
                    out=mask_rep[:, :kc],
                    in_=bass.AP(tensor=m_d[i].tensor, offset=m_d[i].offset,
                                ap=[[0, QC], [1, kc]]))
                vals.append(val_bf)
                masks.append(mask_rep)

            # ---- phase 1: per-chunk transpose + key projection
            keyTs, kprojs = [], []
            for i, kc in enumerate(kcs):
                nkt = math.ceil(kc / P)
                rows = [min(P, kc - kt * P) for kt in range(nkt)]
                key_bf = keys[i]

                keyT_bf = kv.tile([P, HT, kc], BF16, tag="keyT")
                for kt in range(nkt):
                    for t in range(HT):
                        ps = ps_tp.tile([P, P], BF16, tag="ps_t")
                        nc.tensor.transpose(
                            ps[:, :rows[kt]],
                            key_bf[:rows[kt], kt, t * P:(t + 1) * P],
                            ident_bf[:rows[kt], :rows[kt]])
                        nc.scalar.copy(keyT_bf[:, t, kt * P:kt * P + rows[kt]],
                                       ps[:, :rows[kt]])

                kproj = kv.tile([P, HT, kc], F32, tag="kproj")
                for to in range(HT):
                    ps = ps_miscp.tile([P, 512], F32, tag="ps_misc")
                    for ti in range(HT):
                        nc.tensor.matmul(ps[:, :kc],
                                         lhsT=wk_bf[:, ti, to * P:(to + 1) * P],
                                         rhs=keyT_bf[:, ti, :kc],
                                         start=(ti == 0), stop=(ti == HT - 1))
                    nc.vector.tensor_copy(kproj[:, to, :kc], ps[:, :kc])

                keyTs.append(keyT_bf)
                kprojs.append(kproj)

            # ---- phase 2: compute per chunk
            for i, kc in enumerate(kcs):
                nkt = math.ceil(kc / P)
                rows = [min(P, kc - kt * P) for kt in range(nkt)]
                kproj, val_bf, mask_rep = kprojs[i], vals[i], masks[i]

                # features: feat_t[p, j, k] = tanh(kproj[p,t,k] + qproj[p,t,32i+j])
                # DVE broadcast add runs at 1x (step-0 dims block packed-read
                # modes) and DVE+GpSimd can't overlap (shared-port lock halves
                # both), so DVE takes rows [0, QD) in one broadcast TT per
                # h-tile and ACT absorbs the rest via fused tanh(kproj + q-bias)
                # per row (its per-partition bias is a free add)
                QD = QC if kc < 224 else QC - 7
                feats = []
                for t in range(HT):
                    ft = featp.tile([P, QC, kc], BF16, tag="feat")
                    kap = kproj[:, t, :kc]
                    in0 = bass.AP(tensor=kap.tensor, offset=kap.offset,
                                  ap=[kap.ap[0], [0, QD], [1, kc]])
                    qap = qproj[:, t, QC * i:QC * i + QD]
                    in1 = bass.AP(tensor=qap.tensor, offset=qap.offset,
                                  ap=[qap.ap[0], [1, QD], [0, kc]])
                    nc.vector.tensor_add(ft[:, :QD, :kc], in0, in1)
                    nc.scalar.activation(out=ft[:, :QD, :kc], in_=ft[:, :QD, :kc],
                                         func=AF.Tanh)
                    for j in range(QD, QC):
                        nc.scalar.activation(
                            out=ft[:, j, :kc], in_=kproj[:, t, :kc],
                            func=AF.Tanh,
                            bias=qproj[:, t, QC * i + j:QC * i + j + 1])
                    feats.append(ft)

                # scores: batched M<=4 x N<=512 matmuls; rpm rows pack per
                # matmul as concatenated [rpm*kc] output on one PSUM partition,
                # wvZ stacks 4 row-packs on partitions 32g..32g+3. t outer so
                # matmuls chase the tanh tiles; g inner so consecutive matmuls
                # hit different PE col-groups and stream concurrently.
                rpm = min(16, 512 // kc)
                ngroups = math.ceil(QC / (4 * rpm))
                ps_sc = ps_scp.tile([P, 2, 512], F32, tag="ps_sc")
                scores_tmp = sm.tile([P, 2, 512], F32, tag="sctmp")
                scores = sm.tile([QC, kc], F32, tag="scores")
                for t in range(HT):
                    for r in range(4):
                        for g in range(ngroups):
                            j0 = g * 4 * rpm
                            rmax = min(4, math.ceil((QC - j0) / rpm))
                            if r >= rmax:
                                continue
                            ps = ps_sc[32 * g:32 * g + rmax, g % 2, :rpm * kc]
                            nc.tensor.matmul(
                                ps, lhsT=wvZ[:, t, r, :rmax],
                                rhs=feats[t][:, j0 + r * rpm:j0 + (r + 1) * rpm, :kc],
                                start=(t == 0 and r == 0),
                                stop=(t == HT - 1 and r == rmax - 1),
                                tile_position=(0, 32 * g))
                for g in range(ngroups):
                    j0 = g * 4 * rpm
                    rmax = min(4, math.ceil((QC - j0) / rpm))
                    nc.vector.tensor_copy(
                        scores_tmp[32 * g:32 * g + rmax, g % 2, :rpm * kc],
                        ps_sc[32 * g:32 * g + rmax, g % 2, :rpm * kc])
                for g in range(ngroups):
                    j0 = g * 4 * rpm
                    rmax = min(4, math.ceil((QC - j0) / rpm))
                    src = scores_tmp[32 * g:32 * g + rmax, g % 2, :rpm * kc]
                    src = bass.AP(tensor=src.tensor, offset=src.offset,
                                  ap=[src.ap[0], [kc, rpm], [1, kc]])
                    nc.sync.dma_start(out=scores[j0:j0 + rmax * rpm, :kc],
                                       in_=src)

                nc.vector.tensor_add(scores[:, :kc], scores[:, :kc],
                                     mask_rep[:, :kc])

                negmax = sm.tile([QC, 1], F32, tag="negmax")
                nc.vector.reduce_max(out=negmax, in_=scores[:, :kc],
                                     axis=mybir.AxisListType.X, negate=True)
                probs = sm.tile([QC, kc], F32, tag="probs")
                sumexp = sm.tile([QC, 1], F32, tag="sumexp")
                nc.scalar.activation(out=probs[:, :kc], in_=scores[:, :kc],
                                     func=AF.Exp, bias=negmax,
                                     accum_out=sumexp)
                rsum = sm.tile([QC, 1], F32, tag="rsum")
                nc.vector.reciprocal(rsum, sumexp)
                probs_bf = sm.tile([QC, kc], BF16, tag="probsb")
                nc.vector.tensor_scalar_mul(probs_bf[:, :kc], probs[:, :kc], rsum)

                attnT = sm.tile([P, nkt, QC], BF16, tag="attnT")
                for kt in range(nkt):
                    ps_t = ps_tp.tile([P, P], BF16, tag="ps_t")
                    nc.tensor.transpose(ps_t[:rows[kt], :QC],
                                        probs_bf[:, kt * P:kt * P + rows[kt]],
                                        ident_bf[:QC, :QC])
                    nc.vector.tensor_copy(attnT[:rows[kt], kt, :],
                                          ps_t[:rows[kt], :QC])

                ps_c = ps_ctxp.tile([P, 512], F32, tag="ps_c")
                for kt in range(nkt):
                    nc.tensor.matmul(ps_c[:QC, :],
                                     lhsT=attnT[:rows[kt], kt, :],
                                     rhs=val_bf[:rows[kt], kt, :],
                                     start=(kt == 0), stop=(kt == nkt - 1))
                ctx = sm.tile([QC, H], F32, tag="ctx")
                nc.vector.tensor_copy(ctx, ps_c[:QC, :])
                nc.sync.dma_start(out=out_d[i], in_=ctx)

    nc.compile()
    return nc


_CACHE: dict = {}
LAST_RESULT = None


def _get_program(kcs: tuple[int, ...]):
    if kcs not in _CACHE:
        _CACHE[kcs] = _build(kcs)
    return _CACHE[kcs]


def kernel(query, key, value, valid_lens, Wq, Wk, wv):
    query = np.ascontiguousarray(np.asarray(query, dtype=np.float32))
    key = np.ascontiguousarray(np.asarray(key, dtype=np.float32))
    value = np.ascontiguousarray(np.asarray(value, dtype=np.float32))
    Wq = np.ascontiguousarray(np.asarray(Wq, dtype=np.float32))
    Wk = np.ascontiguousarray(np.asarray(Wk, dtype=np.float32))
    wv = np.ascontiguousarray(np.asarray(wv, dtype=np.float32))
    vl = np.asarray(valid_lens).astype(np.int64)

    out = np.empty((B, Q, H), dtype=np.float32)

    # fully-masked batches: reference softmax of an all -1e9 row is uniform
    active = [b for b in range(B) if vl[b] > 0]
    for b in range(B):
        if vl[b] <= 0:
            out[b, :, :] = value[b].mean(axis=0)[None, :]

    if not active:
        return out

    # sort big k-extent first so the heavy chunks schedule early
    kcs_b = {b: min(K, int(math.ceil(vl[b] / 32)) * 32) for b in active}
    order = sorted(active, key=lambda b: -kcs_b[b])
    kcs = tuple(kcs_b[b] for b in order)

    nc = _get_program(kcs)

    shared = {"wq": Wq, "wk": Wk, "wv": wv}
    for i, b in enumerate(order):
        kc = kcs[i]
        shared[f"k{i}"] = np.ascontiguousarray(key[b, :kc, :])
        shared[f"v{i}"] = np.ascontiguousarray(value[b, :kc, :])
        m = np.zeros(kc, dtype=np.float32)
        m[min(vl[b], kc):] = NEG
        shared[f"m{i}"] = m

    in_maps = []
    for ci in range(N_CORES):
        q_rows = np.ascontiguousarray(
            np.stack([query[b, QC * ci:QC * (ci + 1), :] for b in order])
        ).reshape(len(order) * QC, H)
        in_maps.append({**shared, "q": q_rows})

    res = run_bass_kernel_spmd(nc, in_maps, core_ids=list(range(N_CORES)))
    global LAST_RESULT
    LAST_RESULT = res

    for ci in range(N_CORES):
        o = res.results[ci]["out"]
        for i, b in enumerate(order):
            out[b, QC * ci:QC * (ci + 1), :] = o[i]
    return out


# revision 29
# speedup vs baseline: 1.5811x; 1.0576x over previous
"""AdditiveAttention Trainium2 kernel (8 NeuronCores, SPMD data-parallel).

reference:
    q = query @ Wq; k = key @ Wk
    scores[b,q,k] = sum_h wv[h] * tanh(q[b,q,h] + k[b,k,h])   (masked k >= valid_len)
    out = softmax(scores) @ value

Sharding: core i takes q-rows [32i, 32i+32) of EVERY batch, so all 8 cores run an
identical instruction stream (chunk per batch with that batch's k-extent) and the
per-batch valid_len-dependent work (k <= Kc_b, Kc_b = valid_len rounded up to 32)
is split exactly evenly.

Per-core dataflow (per batch chunk; partition dim = hidden):
    DMA-cast q/key/Wq/Wk to bf16, DMA-transpose -> hT layouts
    PE: qT' = Wq^T qT, kT' = Wk^T kT  (bf16)
    DVE: per q-row tensor_scalar_add (kT' + q-col) -> feat tile [128h, 32q*Kc]
    ACT: tanh in-place (bf16), one instruction per (batch, h-tile)
    PE:  scores row = wv^T @ feat[:, j, :]  M=1 matmuls, col-tiled 4-wide so 4
         q-rows land on PSUM partitions {0,32,64,96} -> 4-lane DVE copies
    softmax: reduce_max(negate) -> exp(bias=-max, accum_out=sum) -> reciprocal -> scale
    PE: transpose attn, context = attnT^T @ value (f32)
"""

import contextlib
import ctypes
import math
import sys
import types

if "/opt/trn_rl_repo" not in sys.path:
    sys.path.insert(0, "/opt/trn_rl_repo")

import ml_dtypes
import numpy as np


def _install_ntff_hook():
    """This image's antenv package lacks axon_hooks; inject an equivalent so
    run_bass_kernel_spmd can trace (BASS_TRACE=1) instead of crashing."""
    if "antenv.axon_hooks" in sys.modules:
        return
    mod = types.ModuleType("antenv.axon_hooks")
    _state = {"hook": None}
    mod.set_axon_ntff_profile_hook = lambda h: _state.__setitem__("hook", h)
    mod.get_axon_ntff_profile_hook = lambda: _state["hook"]
    try:
        import antenv

        antenv.axon_hooks = mod
    except ImportError:
        pass
    sys.modules["antenv.axon_hooks"] = mod

    try:
        lib = ctypes.CDLL("/opt/axon/libaxon_pjrt.so")
    except OSError:
        return
    if not hasattr(lib, "axon_start_nrt_profile"):
        return
    lib.axon_start_nrt_profile.argtypes = [ctypes.POINTER(ctypes.c_int64),
                                           ctypes.c_size_t]
    lib.axon_start_nrt_profile.restype = ctypes.c_int64
    lib.axon_stop_nrt_profile.argtypes = [ctypes.c_char_p]
    lib.axon_stop_nrt_profile.restype = ctypes.c_int64

    @contextlib.contextmanager
    def _hook(output_dir, device_ids):
        import jax

        jax.devices()
        if device_ids:
            ids = (ctypes.c_int64 * len(device_ids))(*device_ids)
            rc = lib.axon_start_nrt_profile(ids, len(device_ids))
        else:
            rc = lib.axon_start_nrt_profile(None, 0)
        if rc != 0:
            raise RuntimeError(f"axon_start_nrt_profile rc={rc}")
        try:
            yield
        finally:
            n = lib.axon_stop_nrt_profile(str(output_dir).encode())
            print(f"profile: {n} file(s) written to {output_dir}", file=sys.stderr)

    mod.set_axon_ntff_profile_hook(_hook)


_install_ntff_hook()

import concourse.bass as bass
import concourse.bacc as bacc
import concourse.tile as tile
from concourse import mybir
from concourse.bass_utils import run_bass_kernel_spmd
from concourse.masks import make_identity

F32 = mybir.dt.float32
BF16 = mybir.dt.bfloat16
AF = mybir.ActivationFunctionType

B, Q, K, H = 4, 256, 512 // 2, 512
P = 128
HT = H // P  # 4 h-tiles
N_CORES = 8
QC = Q // N_CORES  # 32 q rows per (core, batch)
NEG = -1e9


def _build(kcs: tuple[int, ...]):
    """Build + compile the SPMD program for the given per-chunk k extents
    (one chunk per active batch, every extent a multiple of 32, <= 256)."""
    nb = len(kcs)
    R = nb * QC  # total q rows per core

    nc = bacc.Bacc("TRN2", target_bir_lowering=False, debug=False,
                   num_devices=N_CORES)

    qt_d = nc.dram_tensor("qt", [H, R], BF16, kind="ExternalInput").ap()
    wq_d = nc.dram_tensor("wq", [H, H], BF16, kind="ExternalInput").ap()
    wk_d = nc.dram_tensor("wk", [H, H], BF16, kind="ExternalInput").ap()
    wv_d = nc.dram_tensor("wv", [H], F32, kind="ExternalInput").ap()
    kt_d, v_d, m_d = [], [], []
    for i, kc in enumerate(kcs):
        kt_d.append(nc.dram_tensor(f"kt{i}", [H, kc], BF16,
                                   kind="ExternalInput").ap())
        v_d.append(nc.dram_tensor(f"v{i}", [kc, H], BF16,
                                  kind="ExternalInput").ap())
        m_d.append(nc.dram_tensor(f"m{i}", [kc], F32, kind="ExternalInput").ap())
    out_d = nc.dram_tensor("out", [nb, QC, H], F32, kind="ExternalOutput").ap()

    kcmax = max(kcs)

    with tile.TileContext(nc) as tc:
        with (
            tc.tile_pool(name="consts", bufs=1) as consts,
            tc.tile_pool(name="kv", bufs=4) as kv,
            tc.tile_pool(name="feat", bufs=6) as featp,
            tc.tile_pool(name="sm", bufs=2) as sm,
            tc.tile_pool(name="ps_sc", bufs=1, space="PSUM") as ps_scp,
            tc.tile_pool(name="ps_misc", bufs=2, space="PSUM") as ps_miscp,
            tc.tile_pool(name="ps_t", bufs=2, space="PSUM") as ps_tp,
            tc.tile_pool(name="ps_ctx", bufs=2, space="PSUM") as ps_ctxp,
        ):
            # all inputs arrive pre-cast/pre-transposed from the host;
            # sync HWDGE streams them straight into their compute layouts
            wk_bf = consts.tile([P, HT, H], BF16)
            nc.sync.dma_start(out=wk_bf, in_=wk_d.rearrange("(t p) o -> p t o", p=P))
            qT_bf = consts.tile([P, HT, R], BF16)
            nc.sync.dma_start(out=qT_bf, in_=qt_d.rearrange("(t p) r -> p t r", p=P))
            wq_bf = consts.tile([P, HT, H], BF16)
            nc.sync.dma_start(out=wq_bf, in_=wq_d.rearrange("(t p) o -> p t o", p=P))

            ident_bf = consts.tile([P, P], BF16)
            make_identity(nc, ident_bf)

            wv_f = consts.tile([P, HT], F32)
            nc.sync.dma_start(out=wv_f, in_=wv_d.rearrange("(t p) -> p t", p=P))
            wv_bf = consts.tile([P, HT], BF16)
            nc.vector.tensor_copy(wv_bf, wv_f)
            # wvZ[:, t, r, :] = wv tile t in column r, zeros elsewhere: an M<=4
            # score matmul with this lhsT adds row-pack r's scores at PSUM
            # partition base+r and zeros into the others (which accumulate away)
            wvZ = consts.tile([P, HT, 4, 4], BF16)
            nc.vector.memset(wvZ, 0.0)
            for t in range(HT):
                for r in range(4):
                    nc.vector.tensor_copy(wvZ[:, t, r, r:r + 1], wv_bf[:, t:t + 1])

            # qproj[p, t_out, r] (f32): h_out = t_out*128+p
            qproj = consts.tile([P, HT, R], F32)
            for to in range(HT):
                ps = ps_miscp.tile([P, 512], F32, tag="ps_misc")
                for ti in range(HT):
                    nc.tensor.matmul(ps[:, :R], lhsT=wq_bf[:, ti, to * P:(to + 1) * P],
                                     rhs=qT_bf[:, ti, :],
                                     start=(ti == 0), stop=(ti == HT - 1))
                nc.vector.tensor_copy(qproj[:, to, :], ps[:, :R])

            # ---- phase 0/1: per-chunk inputs + key projection
            keyTs, kprojs, vals, masks = [], [], [], []
            for i, kc in enumerate(kcs):
                nkt = math.ceil(kc / P)

                keyT_bf = kv.tile([P, HT, kc], BF16, tag="keyT")
                nc.sync.dma_start(out=keyT_bf,
                                  in_=kt_d[i].rearrange("(t p) k -> p t k", p=P))

                kproj = kv.tile([P, HT, kc], F32, tag="kproj")
                for to in range(HT):
                    ps = ps_miscp.tile([P, 512], F32, tag="ps_misc")
                    for ti in range(HT):
                        nc.tensor.matmul(ps[:, :kc],
                                         lhsT=wk_bf[:, ti, to * P:(to + 1) * P],
                                         rhs=keyT_bf[:, ti, :kc],
                                         start=(ti == 0), stop=(ti == HT - 1))
                    nc.vector.tensor_copy(kproj[:, to, :kc], ps[:, :kc])

                val_bf = kv.tile([P, nkt, H], BF16, tag="val")
                for kt in range(nkt):
                    r0 = kt * P
                    rr = min(P, kc - r0)
                    nc.sync.dma_start(out=val_bf[:rr, kt, :],
                                      in_=v_d[i][r0:r0 + rr, :])
                mask_rep = kv.tile([QC, kc], F32, tag="mask")
                nc.gpsimd.dma_start(
                    out=mask_rep[:, :kc],
                    in_=bass.AP(tensor=m_d[i].tensor, offset=m_d[i].offset,
                                ap=[[0, QC], [1, kc]]))
                keyTs.append(keyT_bf)
                kprojs.append(kproj)
                vals.append(val_bf)
                masks.append(mask_rep)

            # ---- phase 2: compute per chunk
            for i, kc in enumerate(kcs):
                nkt = math.ceil(kc / P)
                rows = [min(P, kc - kt * P) for kt in range(nkt)]
                kproj, val_bf, mask_rep = kprojs[i], vals[i], masks[i]

                # features: feat_t[p, j, k] = tanh(kproj[p,t,k] + qproj[p,t,32i+j])
                # DVE broadcast add runs at 1x (step-0 dims block packed-read
                # modes) and DVE+GpSimd can't overlap (shared-port lock halves
                # both), so DVE takes rows [0, QD) in one broadcast TT per
                # h-tile and ACT absorbs the rest via fused tanh(kproj + q-bias)
                # per row (its per-partition bias is a free add)
                QD = QC if kc < 224 else QC - 7
                feats = []
                for t in range(HT):
                    ft = featp.tile([P, QC, kc], BF16, tag="feat")
                    kap = kproj[:, t, :kc]
                    in0 = bass.AP(tensor=kap.tensor, offset=kap.offset,
                                  ap=[kap.ap[0], [0, QD], [1, kc]])
                    qap = qproj[:, t, QC * i:QC * i + QD]
                    in1 = bass.AP(tensor=qap.tensor, offset=qap.offset,
                                  ap=[qap.ap[0], [1, QD], [0, kc]])
                    nc.vector.tensor_add(ft[:, :QD, :kc], in0, in1)
                    nc.scalar.activation(out=ft[:, :QD, :kc], in_=ft[:, :QD, :kc],
                                         func=AF.Tanh)
                    for j in range(QD, QC):
                        nc.scalar.activation(
                            out=ft[:, j, :kc], in_=kproj[:, t, :kc],
                            func=AF.Tanh,
                            bias=qproj[:, t, QC * i + j:QC * i + j + 1])
                    feats.append(ft)

                # scores: batched M<=4 x N<=512 matmuls; rpm rows pack per
                # matmul as concatenated [rpm*kc] output on one PSUM partition,
                # wvZ stacks 4 row-packs on partitions 32g..32g+3. t outer so
                # matmuls chase the tanh tiles; g inner so consecutive matmuls
                # hit different PE col-groups and stream concurrently.
                rpm = min(16, 512 // kc)
                ngroups = math.ceil(QC / (4 * rpm))
                ps_sc = ps_scp.tile([P, 2, 512], F32, tag="ps_sc")
                scores_tmp = sm.tile([P, 2, 512], F32, tag="sctmp")
                scores = sm.tile([QC, kc], F32, tag="scores")
                for t in range(HT):
                    for r in range(4):
                        for g in range(ngroups):
                            j0 = g * 4 * rpm
                            rmax = min(4, math.ceil((QC - j0) / rpm))
                            if r >= rmax:
                                continue
                            ps = ps_sc[32 * g:32 * g + rmax, g % 2, :rpm * kc]
                            nc.tensor.matmul(
                                ps, lhsT=wvZ[:, t, r, :rmax],
                                rhs=feats[t][:, j0 + r * rpm:j0 + (r + 1) * rpm, :kc],
                                start=(t == 0 and r == 0),
                                stop=(t == HT - 1 and r == rmax - 1),
                                tile_position=(0, 32 * g))
                for g in range(ngroups):
                    j0 = g * 4 * rpm
                    rmax = min(4, math.ceil((QC - j0) / rpm))
                    nc.vector.tensor_copy(
                        scores_tmp[32 * g:32 * g + rmax, g % 2, :rpm * kc],
                        ps_sc[32 * g:32 * g + rmax, g % 2, :rpm * kc])
                for g in range(ngroups):
                    j0 = g * 4 * rpm
                    rmax = min(4, math.ceil((QC - j0) / rpm))
                    src = scores_tmp[32 * g:32 * g + rmax, g % 2, :rpm * kc]
                    src = bass.AP(tensor=src.tensor, offset=src.offset,
                                  ap=[src.ap[0], [kc, rpm], [1, kc]])
                    nc.sync.dma_start(out=scores[j0:j0 + rmax * rpm, :kc],
                                       in_=src)

                nc.vector.tensor_add(scores[:, :kc], scores[:, :kc],
                                     mask_rep[:, :kc])

                negmax = sm.tile([QC, 1], F32, tag="negmax")
                nc.vector.reduce_max(out=negmax, in_=scores[:, :kc],
                                     axis=mybir.AxisListType.X, negate=True)
                probs = sm.tile([QC, kc], F32, tag="probs")
                sumexp = sm.tile([QC, 1], F32, tag="sumexp")
                nc.scalar.activation(out=probs[:, :kc], in_=scores[:, :kc],
                                     func=AF.Exp, bias=negmax,
                                     accum_out=sumexp)
                rsum = sm.tile([QC, 1], F32, tag="rsum")
                nc.vector.reciprocal(rsum, sumexp)
                probs_bf = sm.tile([QC, kc], BF16, tag="probsb")
                nc.vector.tensor_scalar_mul(probs_bf[:, :kc], probs[:, :kc], rsum)

                attnT = sm.tile([P, nkt, QC], BF16, tag="attnT")
                for kt in range(nkt):
                    ps_t = ps_tp.tile([P, P], BF16, tag="ps_t")
                    nc.tensor.transpose(ps_t[:rows[kt], :QC],
                                        probs_bf[:, kt * P:kt * P + rows[kt]],
                                        ident_bf[:QC, :QC])
                    nc.vector.tensor_copy(attnT[:rows[kt], kt, :],
                                          ps_t[:rows[kt], :QC])

                ps_c = ps_ctxp.tile([P, 512], F32, tag="ps_c")
                for kt in range(nkt):
                    nc.tensor.matmul(ps_c[:QC, :],
                                     lhsT=attnT[:rows[kt], kt, :],
                                     rhs=val_bf[:rows[kt], kt, :],
                                     start=(kt == 0), stop=(kt == nkt - 1))
                ctx = sm.tile([QC, H], F32, tag="ctx")
                nc.vector.tensor_copy(ctx, ps_c[:QC, :])
                nc.sync.dma_start(out=out_d[i], in_=ctx)

    nc.compile()
    return nc


_CACHE: dict = {}
LAST_RESULT = None


def _get_program(kcs: tuple[int, ...]):
    if kcs not in _CACHE:
        _CACHE[kcs] = _build(kcs)
    return _CACHE[kcs]


def kernel(query, key, value, valid_lens, Wq, Wk, wv):
    query = np.ascontiguousarray(np.asarray(query, dtype=np.float32))
    key = np.ascontiguousarray(np.asarray(key, dtype=np.float32))
    value = np.ascontiguousarray(np.asarray(value, dtype=np.float32))
    Wq = np.ascontiguousarray(np.asarray(Wq, dtype=np.float32))
    Wk = np.ascontiguousarray(np.asarray(Wk, dtype=np.float32))
    wv = np.ascontiguousarray(np.asarray(wv, dtype=np.float32))
    vl = np.asarray(valid_lens).astype(np.int64)

    out = np.empty((B, Q, H), dtype=np.float32)

    # fully-masked batches: reference softmax of an all -1e9 row is uniform
    active = [b for b in range(B) if vl[b] > 0]
    for b in range(B):
        if vl[b] <= 0:
            out[b, :, :] = value[b].mean(axis=0)[None, :]

    if not active:
        return out

    # sort big k-extent first so the heavy chunks schedule early
    kcs_b = {b: min(K, int(math.ceil(vl[b] / 32)) * 32) for b in active}
    order = sorted(active, key=lambda b: -kcs_b[b])
    kcs = tuple(kcs_b[b] for b in order)

    nc = _get_program(kcs)

    bf = ml_dtypes.bfloat16
    shared = {"wq": Wq.astype(bf), "wk": Wk.astype(bf), "wv": wv}
    for i, b in enumerate(order):
        kc = kcs[i]
        shared[f"kt{i}"] = np.ascontiguousarray(key[b, :kc, :].T.astype(bf))
        shared[f"v{i}"] = np.ascontiguousarray(value[b, :kc, :].astype(bf))
        m = np.zeros(kc, dtype=np.float32)
        m[min(vl[b], kc):] = NEG
        shared[f"m{i}"] = m

    in_maps = []
    for ci in range(N_CORES):
        q_rows = np.stack([query[b, QC * ci:QC * (ci + 1), :] for b in order]
                          ).reshape(len(order) * QC, H)
        in_maps.append({**shared,
                        "qt": np.ascontiguousarray(q_rows.T.astype(bf))})

    res = run_bass_kernel_spmd(nc, in_maps, core_ids=list(range(N_CORES)))
    global LAST_RESULT
    LAST_RESULT = res

    for ci in range(N_CORES):
        o = res.results[ci]["out"]
        for i, b in enumerate(order):
            out[b, QC * ci:QC * (ci + 1), :] = o[i]
    return out
